# revision 12
# baseline (speedup 1.0000x reference)
"""Trainium2 Bass kernel for single-query pooling attention.

Reference computation (B=32, N=4096, C=768, H=8, DH=96):
    q = (queries @ Wq.T).reshape(H, DH)
    k/v from x @ Wkv.T ; dots = q.k ; attn = softmax_n(dots)
    out = Wproj(attn-weighted sum of v) + bproj     -> [B, 1, C]

Algebraic reduction (never materializes k/v):
    wk_eff[h,c] = sum_d q[h,d] * Wkv[h*DH+d, c]         (host, tiny)
    dots[n,h]   = x[n,:] @ wk_eff[h,:]                  (PE, from xT)
    w = exp(dots)  (no max subtraction: |dots| <~ 20, safe in f32)
    pooled[h,c] = sum_n w[n,h] x[n,c] ; sumw[h] = sum_n w[n,h]
    z[hd]  = per-head pooled @ Wv.T ; out = z @ Wproj.T + bproj

v2: x is loaded from HBM ONCE (bf16, natural layout, ~25 MB/core vs
50 MB for the v1 dual-layout scheme -- the kernel was at the HBM
roofline).  The xT layout needed by the dots matmul is produced
on-chip: PE transpose per [128,128] chunk into PSUM (bf16), then two
large DVE copies per 512-row tile into SBUF.  The dots and pooled
matmuls are 4-way column-tiled (tile_position=(0,32g)) which gives
~3.6x PE throughput for these M=8 matmuls; group g == sub-tile s so
the dots output, exp, w-transpose and pooled stationary all line up
with zero data rearrangement.  Per-group pooled partials are combined
and normalized on the DVE at end of batch.

Sharding: pure data-parallel over batch, 4 batches per core, 8 cores.
No collectives needed.
"""

import sys

sys.path.insert(0, "/opt/trn_rl_repo")

import numpy as np

import concourse.bass as bass
import concourse.tile as tile
from concourse import bacc, mybir

B, N, C, H = 32, 4096, 768, 8
DH = C // H
N_CORES = 8
B_LOC = B // N_CORES          # 4 batches per core
TILE = 512                    # n rows per tile
SUB = TILE // 128             # 4 sub-tiles of 128 rows
NT = N // TILE                # 8 tiles per batch
CJ = C // 128                 # 6 c-chunks
C2 = C + 2                    # x padded with 2 ones columns (sumw via matmul)
NG = 4                        # col-tile groups (g == sub-tile index s)
NCHUNK = SUB * CJ             # 24 [128,128] chunks per tile


def build_graph():
    cdt = mybir.dt.bfloat16
    f32 = mybir.dt.float32

    nc = bacc.Bacc("TRN2", target_bir_lowering=False, debug=False)

    x_d = nc.declare_dram_parameter(
        "x", [B_LOC, NT // 2, 128, 2 * SUB * C2], cdt, isOutput=False
    )
    wk_d = nc.declare_dram_parameter("wkp", [128, CJ // 2, 2, H], cdt, isOutput=False)
    wv_d = nc.declare_dram_parameter("wvT", [C, C], cdt, isOutput=False)
    wp_d = nc.declare_dram_parameter("wpT", [C, C], cdt, isOutput=False)
    bp_d = nc.declare_dram_parameter("bproj", [C], f32, isOutput=False)
    id_d = nc.declare_dram_parameter("ident", [128, 128], cdt, isOutput=False)
    idf_d = nc.declare_dram_parameter("identf", [128, 128], f32, isOutput=False)
    out_d = nc.declare_dram_parameter("out", [B_LOC, C], f32, isOutput=True)

    EXP = mybir.ActivationFunctionType.Exp

    with tile.TileContext(nc) as tc:
        with (
            tc.tile_pool(name="const", bufs=1) as const,
            tc.tile_pool(name="xp", bufs=8) as xp,
            tc.tile_pool(name="xtp", bufs=5) as xtp,
            tc.tile_pool(name="wp", bufs=5) as wpool,
            tc.tile_pool(name="wtp", bufs=5) as wtp,
            tc.tile_pool(name="small", bufs=2) as small,
            tc.tile_pool(name="ps_xt", bufs=1, space="PSUM") as ps_xt,
            tc.tile_pool(name="ps_dots", bufs=2, space="PSUM") as ps_dots,
            tc.tile_pool(name="ps_wt", bufs=1, space="PSUM") as ps_wt,
            tc.tile_pool(name="ps_acc", bufs=1, space="PSUM") as ps_acc,
        ):
            ident = const.tile([128, 128], cdt)
            nc.sync.dma_start(ident[:, :], id_d[:, :])
            identf = const.tile([128, 128], f32)
            nc.sync.dma_start(identf[:, :], idf_d[:, :])
            wkp = const.tile([128, CJ // 2, 2, H], cdt)
            nc.sync.dma_start(wkp[:, :, :, :], wk_d[:, :, :, :])

            pooled_all = const.tile([H, B_LOC, C], cdt)
            pT = const.tile([128, CJ, B_LOC, H], cdt)
            zT = const.tile([128, CJ, B_LOC], cdt)
            wvT = const.tile([128, CJ, C], cdt)
            wpT = const.tile([128, CJ, C], cdt)
            bias = const.tile([B_LOC, C], f32)

            x_flat = x_d.ap()

            # v3: tiles round-robin ACROSS batches; col-group == batch.
            # Each batch's dots/pooled land in its own 8-row partition slice
            # (32b..32b+8) -> accumulators are complete per batch, no
            # cross-group combine.  512-wide moving operands keep the PE
            # MMs out of the per-instruction-overhead regime, and the 4
            # batches' MMs run concurrently on the 4 array col-groups.
            acc_lo = ps_acc.tile([128, 512], f32, tag="acc_lo")
            acc_hi = ps_acc.tile([128, C2 - 512], f32, tag="acc_hi")

            recip = small.tile([128, 1], f32, tag="recip")

            def finish_batch(b):
                # normalize: pooled = acc / sumw (sumw at ones col 256 of
                # hi); shifted ops: in at 32b.., out at base 0
                nc.vector.reciprocal(
                    recip[32 * b : 32 * b + 8, :],
                    acc_hi[32 * b : 32 * b + 8, 256:257],
                )
                nc.vector.tensor_scalar_mul(
                    pooled_all[:, b, 0:512],
                    acc_lo[32 * b : 32 * b + 8, :],
                    recip[32 * b : 32 * b + 8, :],
                )
                nc.vector.tensor_scalar_mul(
                    pooled_all[:, b, 512:C],
                    acc_hi[32 * b : 32 * b + 8, 0:256],
                    recip[32 * b : 32 * b + 8, :],
                )
                # pooled -> pooledT -> pT[:, :, b, :]
                pT_ps = ps_wt.tile([128, CJ * H], cdt, tag="wt")
                for cj in range(CJ):
                    nc.tensor.transpose(
                        pT_ps[:, cj * H : (cj + 1) * H],
                        pooled_all[:, b, cj * 128 : (cj + 1) * 128],
                        ident[:H, :H],
                    )
                nc.vector.tensor_copy(
                    pT[:, :, b, :],
                    pT_ps[:, 0 : CJ * H].rearrange("p (j h) -> p j h", j=CJ),
                )

            for t2r in range(NT // 2):
                t2s = [t2r for _ in range(B_LOC)]
                x2s = []
                for b in range(B_LOC):
                    # two tiles per DMA; partition p holds rows 4p..4p+3 of
                    # each half (u)
                    x2 = xp.tile([128, 2, SUB, C2], cdt, tag="x")
                    nc.sync.dma_start(
                        out=x2[:, :, :, :],
                        in_=x_flat[b, t2s[b]].rearrange(
                            "p (u s c) -> p u s c", u=2, s=SUB
                        ),
                    )
                    x2s.append(x2)
                    if b == 0 and t2r == 0:
                        # epilogue weights on the gpsimd (SWDGE) queue so
                        # they never head-of-line block x loads
                        nc.gpsimd.dma_start(
                            out=wvT[:, :, :],
                            in_=wv_d.ap().rearrange("(j p) e -> p j e", p=128),
                        )
                        nc.gpsimd.dma_start(
                            out=wpT[:, :, :],
                            in_=wp_d.ap().rearrange("(j p) e -> p j e", p=128),
                        )
                        bp_ap = bp_d.ap()
                        nc.gpsimd.dma_start(
                            out=bias[:, :],
                            in_=bass.AP(
                                tensor=bp_ap.tensor,
                                offset=bp_ap.offset,
                                ap=[[0, B_LOC], [1, C]],
                            ),
                        )

                for u in range(2):
                    dotsr = ps_dots.tile([128, 512], f32, tag="dots")
                    xts, wss, wts = [], [], []
                    # phase 1: transposes (full-width, serialize anyway)
                    for b in range(B_LOC):
                        x_sb = x2s[b]
                        xt_sb = xtp.tile(
                            [128, CJ // 2 * SUB, 128], f32, tag="xt"
                        )
                        for half, (k0, k1) in enumerate([(0, 8), (8, 12)]):
                            ps_x = ps_xt.tile(
                                [128, k1 - k0, 128], f32, tag=f"xt{half}"
                            )
                            for k in range(k0, k1):
                                cjp, s = divmod(k, SUB)
                                xf = x_sb[:, u, s, :].bitcast(f32)
                                nc.tensor.transpose(
                                    ps_x[:, k - k0, :],
                                    xf[:, cjp * 128 : (cjp + 1) * 128],
                                    identf[:, :],
                                )
                            nc.vector.tensor_copy(
                                xt_sb[:, k0:k1, :], ps_x[:, :, :]
                            )
                        xts.append(
                            xt_sb[:, :, :]
                            .rearrange("p k q -> p (k q)")
                            .bitcast(cdt)
                            .rearrange(
                                "p (cjp s q two) -> p cjp two (s q)",
                                cjp=CJ // 2, s=SUB, two=2,
                            )
                        )

                    # phase 2: dots, batch-interleaved so adjacent MMs hit
                    # different array col-groups (4-way concurrency)
                    for cjp in range(CJ // 2):
                        for e in range(2):
                            for b in range(B_LOC):
                                nc.tensor.matmul(
                                    dotsr[32 * b : 32 * b + 8, :],
                                    wkp[:, cjp, e, :],
                                    xts[b][:, cjp, e, :],
                                    start=(cjp == 0 and e == 0),
                                    stop=(cjp == CJ // 2 - 1 and e == 1),
                                    tile_position=(0, 32 * b),
                                    skip_group_check=True,
                                )

                    # phase 3: exp + wT per batch
                    for b in range(B_LOC):
                        w_sb = wpool.tile([8, 512], cdt, tag="w")
                        nc.scalar.activation(
                            w_sb[:, :], dotsr[32 * b : 32 * b + 8, :], EXP
                        )
                        wt_ps = ps_wt.tile([128, SUB, H], cdt, tag="wt")
                        for s in range(SUB):
                            nc.tensor.transpose(
                                wt_ps[:, s, :],
                                w_sb[:, s * 128 : (s + 1) * 128],
                                ident[:H, :H],
                            )
                        wt_sb = wtp.tile([128, SUB, H], cdt, tag="wts")
                        nc.scalar.copy(wt_sb[:, :, :], wt_ps[:, :, :])
                        wts.append(wt_sb)

                    # phase 4: pooled, batch-interleaved
                    for s in range(SUB):
                        for b in range(B_LOC):
                            first = t2r == 0 and u == 0 and s == 0
                            last = (
                                t2r == NT // 2 - 1 and u == 1 and s == SUB - 1
                            )
                            nc.tensor.matmul(
                                acc_lo[32 * b : 32 * b + 8, :],
                                wts[b][:, s, :],
                                x2s[b][:, u, s, 0:512],
                                start=first,
                                stop=last,
                                tile_position=(0, 32 * b),
                                skip_group_check=True,
                            )
                            nc.tensor.matmul(
                                acc_hi[32 * b : 32 * b + 8, :],
                                wts[b][:, s, :],
                                x2s[b][:, u, s, 512:C2],
                                start=first,
                                stop=last,
                                tile_position=(0, 32 * b),
                                skip_group_check=True,
                            )

                    # staggered finish: a batch whose last tile just closed
                    # gets normalized now, overlapped with remaining rounds
                    if u == 1:
                        for b in range(B_LOC):
                            if t2s[b] == NT // 2 - 1:
                                finish_batch(b)

            # ---- end epilogue: z = per-head pooled @ Wv.T ----
            for h in range(H):
                zT_ps = ps_dots.tile([DH, B_LOC], f32, tag="dots")
                for cj in range(CJ):
                    nc.tensor.matmul(
                        zT_ps[:, :],
                        wvT[:, cj, h * DH : (h + 1) * DH],
                        pT[:, cj, :, h],
                        start=(cj == 0),
                        stop=(cj == CJ - 1),
                    )
                # scatter zT_ps rows (global hd = 96h+d) into zT chunks
                done = 0
                while done < DH:
                    g = h * DH + done
                    j, off = g // 128, g % 128
                    take = min(DH - done, 128 - off, 32)
                    nc.vector.tensor_copy(
                        zT[off : off + take, j, :],
                        zT_ps[done : done + take, :],
                    )
                    done += take

            # out = zT.T @ WprojT + bias
            o_lo = ps_acc.tile([B_LOC, 512], f32, tag="acc_lo")
            o_hi = ps_acc.tile([B_LOC, C - 512], f32, tag="acc_hi")
            for cj in range(CJ):
                nc.tensor.matmul(
                    o_lo[:, :],
                    zT[:, cj, :],
                    wpT[:, cj, 0:512],
                    start=(cj == 0),
                    stop=(cj == CJ - 1),
                )
                nc.tensor.matmul(
                    o_hi[:, :],
                    zT[:, cj, :],
                    wpT[:, cj, 512:C],
                    start=(cj == 0),
                    stop=(cj == CJ - 1),
                )
            out_sb = small.tile([B_LOC, C], f32, tag="osb")
            nc.vector.tensor_add(out_sb[:, 0:512], o_lo[:, :], bias[:, 0:512])
            nc.vector.tensor_add(out_sb[:, 512:C], o_hi[:, :], bias[:, 512:C])
            nc.sync.dma_start(out_d[:, :], out_sb[:, :])

    nc.compile()
    return nc


_NC_CACHE = None


def prepare_in_maps(x, queries, Wq, Wkv, Wproj, bproj):
    x = np.asarray(x, dtype=np.float32)
    queries = np.asarray(queries, dtype=np.float32)
    Wq = np.asarray(Wq, dtype=np.float32)
    Wkv = np.asarray(Wkv, dtype=np.float32)
    Wproj = np.asarray(Wproj, dtype=np.float32)
    bproj = np.asarray(bproj, dtype=np.float32)

    import ml_dtypes

    np_cdt = ml_dtypes.bfloat16

    # host-side weight folding (O(C^2), negligible vs O(B*N*C) device work)
    q = (queries @ Wq.T).reshape(H, DH)                     # [H, DH]
    Wk = Wkv[:C].reshape(H, DH, C)                          # [H, DH, C]
    wk_eff = np.einsum("hd,hdc->hc", q, Wk)                 # [H, C]
    wkp = np.zeros((128, CJ // 2, 2, H), dtype=np.float32)  # paired layout
    for p in range(128):
        for j in range(CJ // 2):
            for e in range(2):
                wkp[p, j, e, :] = wk_eff[:, 256 * j + 2 * p + e]
    wkp = wkp.astype(np_cdt)
    wvT = np.ascontiguousarray(Wkv[C:].T).astype(np_cdt)    # [C, C] (c, hd)
    wpT = np.ascontiguousarray(Wproj.T).astype(np_cdt)      # [C, C] (hd, e)
    ident = np.eye(128, dtype=np.float32).astype(np_cdt)
    identf = np.eye(128, dtype=np.float32)

    xb = x.astype(np_cdt)                                   # [B, N, C]
    in_maps = []
    for core in range(N_CORES):
        xc = xb[core * B_LOC : (core + 1) * B_LOC]          # [B_LOC, N, C]
        xs1 = np.empty((B_LOC * N, C2), dtype=np_cdt)
        xs1[:, :C] = xc.reshape(B_LOC * N, C)
        xs1[:, C:] = 1.0
        # packed: [b, t2, p, (u s c)] with row = t2*1024 + u*512 + 4p + s
        xpk = np.ascontiguousarray(
            xs1.reshape(B_LOC, NT // 2, 2, 128, SUB, C2).transpose(
                0, 1, 3, 2, 4, 5
            )
        ).reshape(B_LOC, NT // 2, 128, 2 * SUB * C2)
        in_maps.append(
            {
                "x": xpk,
                "wkp": wkp,
                "wvT": wvT,
                "wpT": wpT,
                "bproj": bproj,
                "ident": ident,
                "identf": identf,
            }
        )
    return in_maps


def kernel(x, queries, Wq, Wkv, Wproj, bproj):
    global _NC_CACHE
    in_maps = prepare_in_maps(x, queries, Wq, Wkv, Wproj, bproj)
    if _NC_CACHE is None:
        _NC_CACHE = build_graph()
    nc = _NC_CACHE

    from concourse.bass_utils import run_bass_kernel_spmd

    res = run_bass_kernel_spmd(nc, in_maps, core_ids=list(range(N_CORES)))
    out = np.stack([res.results[i]["out"] for i in range(N_CORES)])  # [8,4,C]
    return out.reshape(B, 1, C).astype(np.float32)
